# revision 1
# baseline (speedup 1.0000x reference)
"""CustomLSTM Trainium2 kernel, v2 — gate-major (transposed) layout.

Problem: x [64, 1024, 256], LSTM(I=256, H=512), y = h_last @ fc_w.T + fc_b -> [64, 1].

Strategy (data-parallel over batch, 8 cores x 8 sequences):
- Everything lives transposed: state hT/cT are [128 part (H within chunk),
  4 H-chunks x 8 batch]; gate pre-activations land transposed in PSUM
  directly, so there are NO per-step PE transposes and NO per-step DMAs.
- Recurrence step (per gate n in order g,i,f,o):
    psum_n[128, 32] = sum_m V_n[m-chunk].T @ hT[:, m*8:(m+1)*8]   (16 bf16 MMs, N=8)
    gn = psum_n + xprojT[t, n]                                    (DVE [128,32])
    a_n = sigmoid/tanh(gn)                                        (ACT [128,32])
    c = f*c + i*g ; hT = o * tanh(c)  (bf16 out)                  (DVE [128,32])
- xprojT (x @ U + b, transposed to [gate-col, (t,b)]) is precomputed in
  64-step windows via U-stationary bf16 MMs over PE-transposed x tiles,
  evacuated with a fused bias-add (scalar_tensor_tensor), interleaved with
  the recurrence so PE never idles.
- Inputs are uploaded bf16 (halves tunnel transfer); c stays fp32.
"""
import sys

if "/opt/trn_rl_repo" not in sys.path:
    sys.path.insert(0, "/opt/trn_rl_repo")

import numpy as np
import ml_dtypes
from contextlib import ExitStack

import concourse.bass as bass
import concourse.bacc as bacc
import concourse.tile as tile
import concourse.mybir as mybir
from concourse.bass_utils import run_bass_kernel_spmd

F32 = mybir.dt.float32
BF16 = mybir.dt.bfloat16
AF = mybir.ActivationFunctionType
ALU = mybir.AluOpType
NPBF = ml_dtypes.bfloat16

B, T, I, H = 64, 1024, 256, 512
NCORES = 8
BC = B // NCORES            # 8 sequences per core
W = 64                      # steps per xproj window
NWIN = T // W
# gate order in memory: [i, f, o, g]; processing order g first
GATE_G, GATE_I, GATE_F, GATE_O = 3, 0, 1, 2
NORDER = (GATE_G, GATE_I, GATE_F, GATE_O)


def build_program(n_steps=T):
    nc = bacc.Bacc("TRN2", target_bir_lowering=False, debug=False,
                   num_devices=NCORES)

    xc = nc.dram_tensor("xc", [BC, T, I], BF16, kind="ExternalInput")
    V0 = nc.dram_tensor("V0", [128, 4 * H], BF16, kind="ExternalInput")
    V1 = nc.dram_tensor("V1", [128, 4 * H], BF16, kind="ExternalInput")
    V2 = nc.dram_tensor("V2", [128, 4 * H], BF16, kind="ExternalInput")
    V3 = nc.dram_tensor("V3", [128, 4 * H], BF16, kind="ExternalInput")
    Ud = nc.dram_tensor("Ud", [128, 2 * 4 * H], BF16, kind="ExternalInput")
    biasd = nc.dram_tensor("biasd", [1, 4 * H], BF16, kind="ExternalInput")
    onesd = nc.dram_tensor("onesd", [1, W * BC], BF16, kind="ExternalInput")
    eyed = nc.dram_tensor("eyed", [128, 128], BF16, kind="ExternalInput")
    fcwd = nc.dram_tensor("fcwd", [128, 4], BF16, kind="ExternalInput")
    fcbd = nc.dram_tensor("fcbd", [1, BC], F32, kind="ExternalInput")
    y8 = nc.dram_tensor("y8", [1, BC], F32, kind="ExternalOutput")

    n_win = (n_steps + W - 1) // W

    with ExitStack() as ctx:
        tc_ = ctx.enter_context(tile.TileContext(nc))

        consts = ctx.enter_context(tc_.tile_pool(name="consts", bufs=1))
        xstage = ctx.enter_context(tc_.tile_pool(name="xstage", bufs=2))
        xtp = ctx.enter_context(tc_.tile_pool(name="xtp", bufs=2))
        xproj_pool = ctx.enter_context(tc_.tile_pool(name="xproj", bufs=3))
        state_pool = ctx.enter_context(tc_.tile_pool(name="state", bufs=3))
        work_pool = ctx.enter_context(tc_.tile_pool(name="work", bufs=2))

        ps_gate = ctx.enter_context(tc_.tile_pool(name="ps_g", bufs=1, space="PSUM"))
        ps_u = ctx.enter_context(tc_.tile_pool(name="ps_u", bufs=2, space="PSUM"))
        ps_x = ctx.enter_context(tc_.tile_pool(name="ps_x", bufs=2, space="PSUM"))

        # ---- constants: DMA to staging, relay copy so consumers have
        # engine-sem deps instead of DMA-sem deps.
        def relay(dram_ap, shape, dtype, tag):
            st = consts.tile(shape, dtype, tag=f"{tag}_st")
            nc.sync.dma_start(st[:], dram_ap)
            dst = consts.tile(shape, dtype, tag=tag)
            nc.vector.tensor_copy(dst[:], st[:])
            return dst

        V_sb = [relay(d[:], [128, 4 * H], BF16, f"V{n}")
                for n, d in enumerate([V0, V1, V2, V3])]
        U_sb = relay(Ud[:], [128, 2 * 4 * H], BF16, "U")
        bias_sb = relay(biasd[:], [1, 4 * H], BF16, "bias")
        ones_sb = relay(onesd[:], [1, W * BC], BF16, "ones")
        eye_sb = relay(eyed[:], [128, 128], BF16, "eye")
        fcw_sb = relay(fcwd[:], [128, 4], BF16, "fcw")
        fcb_sb = relay(fcbd[:], [1, BC], F32, "fcb")

        # ---- initial state h=0 (bf16), c=0 (f32)
        c_t = state_pool.tile([128, 4 * BC], F32, tag="c")
        nc.vector.memset(c_t[:], 0.0)
        hT = state_pool.tile([128, 4 * BC], BF16, tag="hT")
        nc.vector.memset(hT[:], 0.0)

        # ================= xproj window precompute =================
        # xpT window tile: [128, W*128] f32; col = t_loc*128 + (n*4+k)*8 + b
        xpT_wins = [None] * n_win
        xT_tiles = {}   # (win, c) -> [128, W*8] bf16 tiles (t,b cols)

        def emit_xchunk(win, sub):
            """Load 16 timesteps of x, PE-transpose into xT tiles."""
            t0 = win * W + sub * 16
            x_t = xstage.tile([128, I], BF16, tag="x")
            nc.sync.dma_start(
                x_t[:],
                xc[:, t0:t0 + 16, :].rearrange("b t i -> t b i"))
            for c in range(2):
                pX = ps_x.tile([128, 128], BF16, tag="psx")
                nc.tensor.transpose(pX[:], x_t[:, c * 128:(c + 1) * 128],
                                    eye_sb[:])
                if sub == 0 and (win, c) not in xT_tiles:
                    xt_new = xtp.tile([128, W * BC], BF16, tag=f"xT{c}")
                    xT_tiles[(win, c)] = xt_new
                nc.vector.tensor_copy(
                    xT_tiles[(win, c)][:, sub * 128:(sub + 1) * 128], pX[:])

        def emit_ugroup(win, j):
            """xprojT for gate-chunk j=(n*4+k) over the whole window."""
            n, k = j // 4, j % 4
            if j == 0:
                xpT_new = xproj_pool.tile([128, W * 128], F32, tag="xpT")
                xpT_wins[win] = xpT_new
            xpT = xpT_wins[win]
            pU = ps_u.tile([128, W * BC], F32, tag="psu")
            for c in range(2):
                nc.tensor.matmul(
                    pU[:], U_sb[:, c * 2048 + n * 512 + k * 128:
                                 c * 2048 + n * 512 + (k + 1) * 128],
                    xT_tiles[(win, c)][:], start=(c == 0), stop=False)
            nc.tensor.matmul(
                pU[:], bias_sb[:, j * 128:(j + 1) * 128], ones_sb[:],
                start=False, stop=True)
            # evacuate chunk-major: xpT cols [j*W*8, (j+1)*W*8)
            nc.vector.tensor_copy(xpT[:, j * W * BC:(j + 1) * W * BC], pU[:])

        def emit_window(win):
            if win >= n_win:
                return
            for sub in range(4):
                emit_xchunk(win, sub)
            for j in range(16):
                emit_ugroup(win, j)

        def release_window(win):
            if win < 0:
                return
            xT_tiles.pop((win, 0), None)
            xT_tiles.pop((win, 1), None)
            xpT_wins[win] = None

        # prefetch first two windows
        emit_window(0)
        emit_window(1)

        # ================= recurrence =================
        for t in range(n_steps):
            win, tl = t // W, t % W
            xpT = xpT_wins[win]
            psums = {}
            acts = {}
            new_hT = state_pool.tile([128, 4 * BC], BF16, tag="hT")
            new_c = state_pool.tile([128, 4 * BC], F32, tag="c")

            for n in NORDER:
                ps = ps_gate.tile([128, 4 * BC], F32, tag=f"ps{n}")
                psums[n] = ps
                for k in range(4):
                    sl = slice(k * BC, (k + 1) * BC)
                    wbase = n * 512 + k * 128
                    for m in range(4):
                        nc.tensor.matmul(
                            ps[:, sl],
                            V_sb[m][:, wbase:wbase + 128],
                            hT[:, m * BC:(m + 1) * BC],
                            start=(m == 0), stop=(m == 3))
                gn = work_pool.tile([128, 4 * BC], F32, tag=f"g{n}")
                xsl = xpT[:].rearrange("p (j t b) -> p j t b", j=16, t=W)[
                    :, n * 4:(n + 1) * 4, tl, :]
                nc.vector.tensor_add(
                    gn[:].rearrange("p (k b) -> p k b", b=BC),
                    ps[:].rearrange("p (k b) -> p k b", b=BC), xsl)
                ga = work_pool.tile([128, 4 * BC], F32, tag=f"a{n}")
                nc.scalar.activation(ga[:], gn[:],
                                     AF.Tanh if n == GATE_G else AF.Sigmoid)
                acts[n] = ga
                if n == GATE_I:
                    ig = work_pool.tile([128, 4 * BC], F32, tag="ig")
                    nc.vector.tensor_mul(ig[:], acts[GATE_I][:],
                                         acts[GATE_G][:])
                elif n == GATE_F:
                    fcx = work_pool.tile([128, 4 * BC], F32, tag="fcx")
                    nc.vector.tensor_mul(fcx[:], acts[GATE_F][:], c_t[:])
                    nc.vector.tensor_add(new_c[:], ig[:], fcx[:])
                    tc_tile = work_pool.tile([128, 4 * BC], F32, tag="tanhc")
                    nc.scalar.activation(tc_tile[:], new_c[:], AF.Tanh)
                elif n == GATE_O:
                    nc.vector.tensor_mul(new_hT[:], acts[GATE_O][:],
                                         tc_tile[:])

            hT, c_t = new_hT, new_c

            # interleave next-window prefetch across this window's steps
            if tl in (4, 8, 12, 16):
                nxt = win + 2
                if nxt < n_win:
                    emit_xchunk(nxt, tl // 4 - 1)
            elif 20 <= tl < 52 and tl % 2 == 0:
                nxt = win + 2
                if nxt < n_win:
                    emit_ugroup(nxt, (tl - 20) // 2)

        # ---- final FC: y = fcw.T-reduced @ hT + fc_b
        ps_fc = ps_u.tile([1, BC], F32, tag="psu")
        for k in range(4):
            nc.tensor.matmul(ps_fc[:], fcw_sb[:, k:k + 1],
                             hT[:, k * BC:(k + 1) * BC],
                             start=(k == 0), stop=(k == 3))
        y_sb = consts.tile([1, BC], F32, tag="y")
        nc.vector.tensor_add(y_sb[:], ps_fc[:], fcb_sb[:])
        nc.sync.dma_start(y8[:], y_sb[:])

    nc.compile()
    return nc


def prep_inputs(x, U_i, V_i, b_i, U_f, V_f, b_f, U_h, V_h, b_h, U_o, V_o, b_o,
                fc_w, fc_b):
    # gate order [i, f, o, g]; g == reference's "h" gate
    f32 = np.float32
    Us = [np.asarray(a, f32) for a in (U_i, U_f, U_o, U_h)]
    Vs = [np.asarray(a, f32) for a in (V_i, V_f, V_o, V_h)]
    bs = [np.asarray(a, f32) for a in (b_i, b_f, b_o, b_h)]

    # V_sb[m]: [128, 4H] bf16 with V_sb[m][q, n*512 + hc] = V_n[m*128+q, hc]
    V_cat = np.concatenate(Vs, axis=1)                      # [512, 2048]
    V_sb = np.ascontiguousarray(
        V_cat.reshape(4, 128, 4 * H)).astype(NPBF)          # [4][128, 2048]

    # U_sb: [128, 2*2048] with U_sb[q, c*2048 + gc] = U_cat[c*128+q, gc]
    U_cat = np.concatenate(Us, axis=1)                      # [256, 2048]
    U_sb = np.ascontiguousarray(
        U_cat.reshape(2, 128, 4 * H).transpose(1, 0, 2).reshape(128, -1)
    ).astype(NPBF)

    b_cat = np.concatenate(bs)                              # [2048]
    bias_sb = np.ascontiguousarray(b_cat[None, :]).astype(NPBF)  # [1, 2048]

    fcw = np.ascontiguousarray(
        np.asarray(fc_w, f32).reshape(4, 128).T).astype(NPBF)
    fcb = np.full((1, BC), float(np.asarray(fc_b).reshape(-1)[0]), f32)

    xb = np.asarray(x, f32).astype(NPBF)                    # one-pass cast

    shared = {
        "V0": V_sb[0], "V1": V_sb[1], "V2": V_sb[2], "V3": V_sb[3],
        "Ud": U_sb, "biasd": bias_sb,
        "onesd": np.ones((1, W * BC), dtype=NPBF),
        "eyed": np.eye(128, dtype=NPBF),
        "fcwd": fcw, "fcbd": fcb,
    }
    in_maps = []
    for c in range(NCORES):
        m = dict(shared)
        m["xc"] = xb[c * BC:(c + 1) * BC]
        in_maps.append(m)
    return in_maps


_CACHED = {}


def kernel(**inputs) -> np.ndarray:
    in_maps = prep_inputs(**inputs)
    if "nc" not in _CACHED:
        _CACHED["nc"] = build_program()
    nc = _CACHED["nc"]
    res = run_bass_kernel_spmd(nc, in_maps, core_ids=list(range(NCORES)))
    out = np.empty((B, 1), np.float32)
    for c in range(NCORES):
        out[c * BC:(c + 1) * BC, 0] = res.results[c]["y8"][0]
    return out



# revision 17
# speedup vs baseline: 10.6999x; 10.6999x over previous
"""CustomLSTM Trainium2 kernel, v3 — lean gate-major recurrence.

Problem: x [64, 1024, 256], LSTM(I=256, H=512), y = h_last @ fc_w.T + fc_b -> [64, 1].
Data-parallel over batch: 8 cores x 8 sequences.

Changes vs v2:
- x uploaded pre-transposed (host does the [B,T,I] -> [I, T*B] shuffle), so the
  per-window xproj precompute is just 2 DMAs + 32 U-matmuls + 16 DVE
  evacuations (bias folded in via tensor_scalar with a per-partition bias
  vector). No PE transposes, no staging copies, no bias matmuls.
- Gate psum seeded with the xproj slice by an eye-matmul (start=True), so the
  ACT engine reads gate pre-activations straight from PSUM; no DVE add.
- Per-step elementwise chain batched: one sigmoid over [128,48] (i,f,o), one
  tanh [128,16] (g), 3 DVE ops for the c update, one tanh, one DVE mul for h.
  Processed in 2 half-steps (h-chunks 0,1 | 2,3) so the chain of half k
  overlaps the PE matmuls of half k+1 and of the next step.
- fp16 weights/activations (better mantissa than bf16 at identical PE cost),
  fp32 psum accumulation and cell state.
"""
import sys

if "/opt/trn_rl_repo" not in sys.path:
    sys.path.insert(0, "/opt/trn_rl_repo")

import numpy as np
from contextlib import ExitStack

import concourse.bass as bass
import concourse.bacc as bacc
import concourse.tile as tile
import concourse.mybir as mybir
from concourse.bass_utils import run_bass_kernel_spmd

F32 = mybir.dt.float32
F16 = mybir.dt.float16
AF = mybir.ActivationFunctionType
NPF16 = np.float16

B, T, I, H = 64, 1024, 256, 512
NCORES = 8
BC = B // NCORES            # 8 sequences per core
W = 64                      # steps per xproj window
NWIN = T // W
# gate processing order in the psum bank: [i, f, o, g]
# j-chunk index: j = gate*4 + m_out  (m_out = output h-chunk 0..3)


def build_program(n_steps=T, mode="full"):
    nc = bacc.Bacc("TRN2", target_bir_lowering=False, debug=False,
                   num_devices=NCORES)

    xcT = nc.dram_tensor("xcT", [128, 2 * T * BC], F16, kind="ExternalInput")
    Vw = nc.dram_tensor("Vw", [128, 16 * 4 * 128], F16, kind="ExternalInput")
    Uw = nc.dram_tensor("Uw", [128, 2 * 16 * 128], F16, kind="ExternalInput")
    biasd = nc.dram_tensor("biasd", [128, 16], F32, kind="ExternalInput")
    eyed = nc.dram_tensor("eyed", [128, 128], F16, kind="ExternalInput")
    fcwd = nc.dram_tensor("fcwd", [128, 4], F16, kind="ExternalInput")
    fcbd = nc.dram_tensor("fcbd", [1, BC], F32, kind="ExternalInput")
    y8 = nc.dram_tensor("y8", [1, BC], F32, kind="ExternalOutput")
    dbgd = None
    if mode == "nochain":
        dbgd = nc.dram_tensor("dbg", [128, 64], F32, kind="ExternalOutput")

    n_win = (n_steps + W - 1) // W

    with ExitStack() as ctx:
        tc_ = ctx.enter_context(tile.TileContext(nc))

        consts = ctx.enter_context(tc_.tile_pool(name="consts", bufs=1))
        xtp = ctx.enter_context(tc_.tile_pool(name="xtp", bufs=2))
        xpp = ctx.enter_context(tc_.tile_pool(name="xpp", bufs=3))
        state = ctx.enter_context(tc_.tile_pool(name="state", bufs=2))
        work = ctx.enter_context(tc_.tile_pool(name="work", bufs=2))

        ps_gate = ctx.enter_context(tc_.tile_pool(name="psg", bufs=2, space="PSUM"))
        ps_u = ctx.enter_context(tc_.tile_pool(name="psu", bufs=2, space="PSUM"))

        # ---- constants: DMA to staging, relay-copy so consumers get
        # engine-sem deps instead of DMA-sem deps.
        def relay(dram_ap, shape, dtype, tag):
            st = consts.tile(shape, dtype, tag=f"{tag}_st")
            nc.sync.dma_start(st[:], dram_ap)
            dst = consts.tile(shape, dtype, tag=tag)
            nc.vector.tensor_copy(dst[:], st[:])
            return dst

        Vw_sb = relay(Vw[:], [128, 16 * 4 * 128], F16, "Vw")
        Uw_sb = relay(Uw[:], [128, 2 * 16 * 128], F16, "Uw")
        bias_sb = relay(biasd[:], [128, 16], F32, "bias")
        eye_sb = relay(eyed[:], [128, 128], F16, "eye")
        fcw_sb = relay(fcwd[:], [128, 4], F16, "fcw")
        fcb_sb = relay(fcbd[:], [1, BC], F32, "fcb")

        # ---- initial state: h halves [128, 16] f16, c halves [128, 16] f32
        h_prev = []
        c_prev = []
        for hf in range(2):
            ht = state.tile([128, 2 * BC], F16, tag=f"h{hf}")
            nc.vector.memset(ht[:], 0.0)
            h_prev.append(ht)
            ct = state.tile([128, 2 * BC], F32, tag=f"c{hf}")
            nc.vector.memset(ct[:], 0.0)
            c_prev.append(ct)

        # ================= xproj window machinery =================
        # xpT window layout: [128, t(64) x hf(2) x gate(4) x m_loc(2) x b(8)]
        xpT_wins = [None] * n_win
        xt_tiles = {}           # (win, c) -> [128, W*BC] tile

        def emit_xdma(win, c):
            t0 = win * W
            xt = xtp.tile([128, W * BC], F16, tag=f"xt{c}")
            xt_tiles[(win, c)] = xt
            src = xcT[:].rearrange("p (c t b) -> p c t b", c=2, t=T)[
                :, c, t0:t0 + W, :]
            nc.sync.dma_start(
                xt[:].rearrange("p (t b) -> p t b", t=W), src)

        def emit_ugroup(win, j):
            """xprojT for j-chunk j = gate*4+m_out, whole window."""
            if j == 0:
                xpT_new = xpp.tile([128, W * 128], F16, tag="xp")
                xpT_wins[win] = xpT_new
            xpT = xpT_wins[win]
            pu = ps_u.tile([128, W * BC], F32, tag="pu")
            for c in range(2):
                nc.tensor.matmul(
                    pu[:], Uw_sb[:, (c * 16 + j) * 128:(c * 16 + j + 1) * 128],
                    xt_tiles[(win, c)][:], start=(c == 0), stop=(c == 1))
            gate, m_out = j // 4, j % 4
            hf, m_loc = m_out // 2, m_out % 2
            dst = xpT[:].rearrange(
                "p (t hf g m b) -> p t hf g m b", t=W, hf=2, g=4, m=2)[
                :, :, hf, gate, m_loc, :]
            # evacuate on ACT with fused per-partition bias add, keeping the
            # DVE free for the recurrence-critical c/h ops
            nc.scalar.activation(
                dst, pu[:].rearrange("p (t b) -> p t b", t=W),
                AF.Identity, bias=bias_sb[:, j:j + 1])

        def release_window(win):
            if win < 0:
                return
            xt_tiles.pop((win, 0), None)
            xt_tiles.pop((win, 1), None)
            if win < n_win:
                xpT_wins[win] = None

        # prologue: first two windows
        for wv in (0, 1):
            if wv < n_win:
                emit_xdma(wv, 0)
                emit_xdma(wv, 1)
                for j in range(16):
                    emit_ugroup(wv, j)

        # ================= recurrence =================
        dbg_tile = None
        for t in range(n_steps):
            win, tl = t // W, t % W
            xpT = xpT_wins[win]
            new_h = [None, None]
            new_c = [None, None]
            for hf in range(2):
                gp = ps_gate.tile([128, 64], F32, tag=f"gp{hf}")
                # seed psum with xproj(+bias) slice via eye-matmul
                base = tl * 128 + hf * 64
                nc.tensor.matmul(gp[:], eye_sb[:],
                                 xpT[:, base:base + 64],
                                 start=True, stop=(mode == "novmm"))
                if mode != "novmm":
                    for m_src in range(4):
                        rhs = h_prev[m_src // 2][
                            :, (m_src % 2) * BC:(m_src % 2 + 1) * BC]
                        for gate in range(4):
                            for m_loc in range(2):
                                j = gate * 4 + hf * 2 + m_loc
                                nc.tensor.matmul(
                                    gp[:, gate * 16 + m_loc * 8:
                                       gate * 16 + m_loc * 8 + 8],
                                    Vw_sb[:, (j * 4 + m_src) * 128:
                                          (j * 4 + m_src + 1) * 128],
                                    rhs,
                                    start=False,
                                    stop=(m_src == 3 and gate == 3
                                          and m_loc == 1))

                if mode == "nochain":
                    dbg_tile = work.tile([128, 64], F32, tag=f"dmy{hf}")
                    nc.vector.tensor_copy(dbg_tile[:], gp[:])
                    continue

                # ---- gate activations for this half (both halves' gate
                # acts are emitted before any tanh(c): ACT never stalls)
                S = work.tile([128, 48], F32, tag=f"S{hf}")
                nc.scalar.activation(S[:], gp[:, 0:48], AF.Sigmoid)
                G = work.tile([128, 16], F32, tag=f"G{hf}")
                nc.scalar.activation(G[:], gp[:, 48:64], AF.Tanh)
                new_h[hf] = (S, G)

            if mode != "nochain":
                # ---- c update (DVE), overlaps the other half's gate acts
                tcs = [None, None]
                for hf in range(2):
                    S, G = new_h[hf]
                    fc = work.tile([128, 16], F32, tag=f"fc{hf}")
                    nc.vector.tensor_mul(fc[:], S[:, 16:32], c_prev[hf][:])
                    ig = work.tile([128, 16], F32, tag=f"ig{hf}")
                    nc.vector.tensor_mul(ig[:], S[:, 0:16], G[:])
                    cn = state.tile([128, 16], F32, tag=f"c{hf}")
                    nc.vector.tensor_add(cn[:], ig[:], fc[:])
                    new_c[hf] = cn
                for hf in range(2):
                    tc3 = work.tile([128, 16], F32, tag=f"tc{hf}")
                    nc.scalar.activation(tc3[:], new_c[hf][:], AF.Tanh)
                    tcs[hf] = tc3
                for hf in range(2):
                    S, _ = new_h[hf]
                    hn = state.tile([128, 16], F16, tag=f"h{hf}")
                    nc.vector.tensor_mul(hn[:], S[:, 32:48], tcs[hf][:])
                    new_h[hf] = hn
                h_prev = new_h
                c_prev = new_c

            # interleave next-window construction across this window's steps
            nxt = win + 2
            if nxt < n_win:
                if tl == 0:
                    emit_xdma(nxt, 0)
                elif tl == 2:
                    emit_xdma(nxt, 1)
                elif 8 <= tl < 40 and tl % 2 == 0:
                    emit_ugroup(nxt, (tl - 8) // 2)
                elif tl == 40:
                    release_window(win - 1)

        # ---- final FC: y = fc_w-reduced over h + fc_b
        ps_fc = ps_u.tile([1, BC], F32, tag="fcp")
        for m in range(4):
            nc.tensor.matmul(ps_fc[:], fcw_sb[:, m:m + 1],
                             h_prev[m // 2][:, (m % 2) * BC:(m % 2 + 1) * BC],
                             start=(m == 0), stop=(m == 3))
        y_sb = consts.tile([1, BC], F32, tag="y")
        nc.vector.tensor_add(y_sb[:], ps_fc[:], fcb_sb[:])
        nc.sync.dma_start(y8[:], y_sb[:])
        if mode == "nochain" and dbg_tile is not None:
            nc.sync.dma_start(dbgd[:], dbg_tile[:])

    nc.compile()
    return nc


def prep_inputs(x, U_i, V_i, b_i, U_f, V_f, b_f, U_h, V_h, b_h, U_o, V_o, b_o,
                fc_w, fc_b):
    # gate order in the kernel: [i, f, o, g];  g == reference's "h" gate
    f32 = np.float32
    Us = [np.asarray(a, f32) for a in (U_i, U_f, U_o, U_h)]
    Vs = [np.asarray(a, f32) for a in (V_i, V_f, V_o, V_h)]
    bs = [np.asarray(a, f32) for a in (b_i, b_f, b_o, b_h)]

    # Vw[p, (j*4+m_src)*128 + q] = V_gate[m_src*128+p, m_out*128+q], j = gate*4+m_out
    Vw = np.empty((128, 16 * 4 * 128), np.float32)
    for gate in range(4):
        Vg = Vs[gate]
        for m_out in range(4):
            j = gate * 4 + m_out
            for m_src in range(4):
                blk = Vg[m_src * 128:(m_src + 1) * 128,
                         m_out * 128:(m_out + 1) * 128]
                Vw[:, (j * 4 + m_src) * 128:(j * 4 + m_src + 1) * 128] = blk
    # Uw[p, (c*16+j)*128 + q] = U_gate[c*128+p, m_out*128+q]
    Uw = np.empty((128, 2 * 16 * 128), np.float32)
    for gate in range(4):
        Ug = Us[gate]
        for m_out in range(4):
            j = gate * 4 + m_out
            for c in range(2):
                blk = Ug[c * 128:(c + 1) * 128,
                         m_out * 128:(m_out + 1) * 128]
                Uw[:, (c * 16 + j) * 128:(c * 16 + j + 1) * 128] = blk
    # biasd[q, j] = b_gate[m_out*128+q]
    biasb = np.empty((128, 16), np.float32)
    for gate in range(4):
        for m_out in range(4):
            biasb[:, gate * 4 + m_out] = bs[gate][m_out * 128:(m_out + 1) * 128]

    fcw = np.ascontiguousarray(
        np.asarray(fc_w, f32).reshape(4, 128).T).astype(NPF16)
    fcb = np.full((1, BC), float(np.asarray(fc_b).reshape(-1)[0]), f32)

    # x -> per-core transposed fp16: xcT[p, c, t, b] = x[b, t, c*128+p]
    xf = np.asarray(x, f32).astype(NPF16)        # [B, T, I]

    shared = {
        "Vw": np.ascontiguousarray(Vw).astype(NPF16),
        "Uw": np.ascontiguousarray(Uw).astype(NPF16),
        "biasd": biasb,
        "eyed": np.eye(128, dtype=NPF16),
        "fcwd": fcw, "fcbd": fcb,
    }
    in_maps = []
    for core in range(NCORES):
        xc = xf[core * BC:(core + 1) * BC]       # [8, T, I]
        xt = xc.transpose(2, 1, 0)               # [I, T, 8]
        xt = xt.reshape(2, 128, T, BC).transpose(1, 0, 2, 3).reshape(128, -1)
        m = dict(shared)
        m["xcT"] = np.ascontiguousarray(xt)
        in_maps.append(m)
    return in_maps


_CACHED = {}


def kernel(**inputs) -> np.ndarray:
    in_maps = prep_inputs(**inputs)
    if "nc" not in _CACHED:
        _CACHED["nc"] = build_program()
    nc = _CACHED["nc"]
    res = run_bass_kernel_spmd(nc, in_maps, core_ids=list(range(NCORES)))
    out = np.empty((B, 1), np.float32)
    for c in range(NCORES):
        out[c * BC:(c + 1) * BC, 0] = res.results[c]["y8"][0]
    return out
